# revision 4
# baseline (speedup 1.0000x reference)
"""BSplineKAN layer forward on 8 Trainium2 NeuronCores (Bass/Tile).

out = silu(x @ base_weight) + einsum('bir,ior->bo', bspline_basis(x), coeff)

Math: with uniform knots t_j = t3 + (j-3)*h (t3 = left clamp bound) and
s = clamp(x*inv_h + s_bias, 0, G), the cubic B-spline basis functions are
exact linear combinations of 8 one-sided cubic features of s:
  L_k = max(s-k,0)*(s-k)^2 , R_k = min(s-k,0)*(s-k)^2   (k=1..4).
The (features -> basis) map M is solved on the host in float64 and folded
into the coeff tensor:  W2[(f,i), o] = sum_r M[f,r] * coeff[i,o,r].

Host also precomputes sT = transpose(x*inv_h + s_bias) in f16, so the
device needs no transposes or casts: the resident sT tiles serve directly
as (a) the base-matmul stationary operand (base weights are pre-scaled by
h, with the -h*s_bias*sum_i wb[i,o] constant added via a contraction-1
ones-row matmul so it lands inside the silu), and (b) the feature source
after one clamp op.

Per 512-column window the features are built with cheap f16 ops spread
over the three elementwise engines (DVE ts/tt, ACT Square, Pool ts), and
the TensorE does 37 f16 matmuls per 128-row batch tile (32 spline chunks,
4 base chunks, 1 const row). TensorE is the bottleneck at ~31us/window.

Sharding: data-parallel over batch; each of 8 cores handles 2048 rows with
replicated weights. No collectives needed.
"""

import numpy as np

import concourse.bass as bass
import concourse.mybir as mybir
import concourse.tile as tile
from concourse import bacc
from concourse.bass_utils import run_bass_kernel_spmd

N_CORES = 8
BATCH, N_IN, N_OUT = 16384, 512, 512
SPLINE_ORDER, N_GRID = 3, 5
N_BASIS = N_GRID + SPLINE_ORDER  # 8
B_CORE = BATCH // N_CORES        # 2048
N_IC = N_IN // 128               # 4 contraction chunks per feature
N_FEAT = 8
WINDOW = 512
N_W = B_CORE // WINDOW           # 4
N_BT = WINDOW // 128             # 4

f32 = mybir.dt.float32
f16 = mybir.dt.float16
AF = mybir.ActivationFunctionType
ALU = mybir.AluOpType


# ----------------------------------------------------------------------------
# Host-side math
# ----------------------------------------------------------------------------

def _bspline_basis_f64(x, knots):
    """Cox-de Boor recursion (float64), matching the reference semantics."""
    t = np.asarray(knots, np.float64)
    xc = np.clip(np.asarray(x, np.float64),
                 t[SPLINE_ORDER], t[-SPLINE_ORDER - 1])[..., None]
    n_int = len(t) - 1
    B = ((xc >= t[:-1]) & (xc < t[1:])).astype(np.float64)
    for j in range(1, SPLINE_ORDER + 1):
        nv = n_int - j
        ti = t[:nv]
        ti_j = t[j:nv + j]
        ti1 = t[1:nv + 1]
        ti_j1 = t[j + 1:nv + j + 1]
        a1 = (xc - ti) / np.maximum(ti_j - ti, 1e-8)
        a2 = (ti_j1 - xc) / np.maximum(ti_j1 - ti1, 1e-8)
        B = a1 * B[..., :nv] + a2 * B[..., 1:nv + 1]
    return B  # (..., N_BASIS)


def _features_f64(s):
    """One-sided cubes of s (float64). Returns (..., 8)."""
    F = []
    for k in range(1, 5):
        d = s - k
        q = d * d
        F.append(np.maximum(d, 0.0) * q)   # L_k
        F.append(np.minimum(d, 0.0) * q)   # R_k
    return np.stack(F, axis=-1)


def _solve_basis_map(knots):
    """M (8 x 8) with basis = features @ M, solved in f64."""
    t3 = float(knots[SPLINE_ORDER])
    h = float(knots[SPLINE_ORDER + 1] - knots[SPLINE_ORDER])
    g = np.linspace(t3 - 0.5, t3 + N_GRID * h + 0.5, 4001)
    g = np.concatenate([g, np.asarray(knots, np.float64),
                        [t3, t3 + N_GRID * h]])
    sg = (np.clip(g, t3, t3 + N_GRID * h) - t3) / h
    F = _features_f64(sg)
    Bref = _bspline_basis_f64(g, knots)
    M, _, _, _ = np.linalg.lstsq(F, Bref, rcond=None)
    err = np.abs(F @ M - Bref).max()
    # knots come in as float32 and are not exactly uniform, so the closed-form
    # uniform features reproduce the reference basis only to ~1e-7.
    if err > 1e-5:
        raise ValueError(f"basis map residual too large: {err}")
    return M, t3, h


def _prepare_weights(coeff, base_weight, knots):
    """Returns (w2[32,128,512]f16, wb[4,128,512]f16, crow[1,512]f16,
    inv_h, s_bias, s_hi)."""
    M, t3, h = _solve_basis_map(np.asarray(knots, np.float64))
    M = M.copy()
    M[1::2] = -M[1::2]   # device computes fR' = relu(-d)*d^2 = -R_k
    c64 = np.asarray(coeff, np.float64)                      # (i, o, r)
    w2 = np.einsum("fr,ior->fio", M, c64)                    # (8, 512, 512)
    w2_16 = w2.astype(np.float32).astype(np.float16)
    # chunk order [fi*N_IC + ic] -> [32, 128, 512]
    w2_dev = np.ascontiguousarray(
        w2_16.reshape(N_FEAT, N_IC, 128, N_OUT).reshape(-1, 128, N_OUT))
    bw64 = np.asarray(base_weight, np.float64)
    wb_dev = np.ascontiguousarray(
        (h * bw64).astype(np.float32).astype(np.float16)
        .reshape(N_IC, 128, N_OUT))
    inv_h = 1.0 / h
    s_bias = -t3 / h
    s_hi = float((float(knots[-SPLINE_ORDER - 1]) - t3) / h)
    return w2_dev, wb_dev, float(inv_h), float(s_bias), s_hi


def _prepare_x(x, inv_h):
    """Full x (16384, 512) f32 -> per-core s'T arrays [4, 128, 2048] f16,
    with s' = x/h (the -t3/h shift is folded into the device constants so
    the base matmul needs no constant row)."""
    s = (np.asarray(x, np.float32) * np.float32(inv_h)).astype(np.float16)
    # (B, I) -> cores (c, b, i) -> [c][ic, p, b]
    s = s.reshape(N_CORES, B_CORE, N_IC, 128)
    sT = np.ascontiguousarray(s.transpose(0, 2, 3, 1))  # (8, 4, 128, 2048)
    return [sT[c] for c in range(N_CORES)]


# ----------------------------------------------------------------------------
# Device kernel (one SPMD program, run on all 8 cores)
# ----------------------------------------------------------------------------

def _build_nc(s_bias, s_hi, loop_n=None):
    """Device tensors hold s' = s - s_bias; clamp bounds and the d_k
    offsets absorb the shift. loop_n wraps the body in a hardware For_i
    loop (idempotent) for delta-timing."""
    nc = bacc.Bacc()
    sT_ext = nc.declare_dram_parameter("sT", [N_IC, 128, B_CORE], f16,
                                       isOutput=False)
    w2_ext = nc.declare_dram_parameter("w2", [N_FEAT * N_IC, 128, N_OUT], f16,
                                       isOutput=False)
    wb_ext = nc.declare_dram_parameter("wb", [N_IC, 128, N_OUT], f16,
                                       isOutput=False)
    out_ext = nc.declare_dram_parameter("out", [B_CORE, N_OUT], f32,
                                        isOutput=True)

    with tile.TileContext(nc) as tc:
        with tc.tile_pool(name="wpool", bufs=1) as wpool, \
             tc.tile_pool(name="spool", bufs=2) as spool, \
             tc.tile_pool(name="xcpool", bufs=2) as xcpool, \
             tc.tile_pool(name="fpool", bufs=2) as fpool, \
             tc.tile_pool(name="tmp", bufs=3) as tmp, \
             tc.tile_pool(name="opool", bufs=3) as opool, \
             tc.tile_pool(name="psum_b", bufs=4, space="PSUM") as psum_b, \
             tc.tile_pool(name="psum_s", bufs=4, space="PSUM") as psum_s:

            # resident weights (outside the timing loop, loaded once)
            w2_tiles = []
            for ci in range(N_FEAT * N_IC):
                t = wpool.tile([128, N_OUT], f16, tag=f"w2_{ci}")
                nc.sync.dma_start(out=t[:], in_=w2_ext[ci])
                w2_tiles.append(t)
            wb_tiles = []
            for ic in range(N_IC):
                t = wpool.tile([128, N_OUT], f16, tag=f"wb_{ic}")
                nc.sync.dma_start(out=t[:], in_=wb_ext[ic])
                wb_tiles.append(t)
            import contextlib
            loop_cm = (tc.For_i(0, loop_n, 1) if loop_n
                       else contextlib.nullcontext())
            with loop_cm:
                sT_tiles = []
                for ic in range(N_IC):
                    t = spool.tile([128, B_CORE], f16, tag=f"sT_{ic}")
                    nc.sync.dma_start(out=t[:], in_=sT_ext[ic])
                    sT_tiles.append(t)

                for w in range(N_W):
                    cs = slice(w * WINDOW, (w + 1) * WINDOW)
                    feat = {}
                    xcs = []
                    for ic in range(N_IC):
                        xc = xcpool.tile([128, WINDOW], f16, tag=f"xc_{ic}")
                        nc.vector.tensor_scalar(
                            out=xc[:], in0=sT_tiles[ic][:, cs],
                            scalar1=-s_bias, scalar2=s_hi - s_bias,
                            op0=ALU.max, op1=ALU.min)
                        xcs.append(xc)
                    for k in range(1, 5):
                        for ic in range(N_IC):
                            d = tmp.tile([128, WINDOW], f16, tag="d")
                            nc.vector.tensor_scalar(
                                out=d[:], in0=xcs[ic][:],
                                scalar1=float(k) - s_bias, scalar2=None,
                                op0=ALU.subtract)
                            q = tmp.tile([128, WINDOW], f16, tag="q")
                            nc.scalar.activation(q[:], d[:], AF.Square)
                            u = tmp.tile([128, WINDOW], f16, tag="u")
                            nc.vector.tensor_scalar(
                                out=u[:], in0=d[:],
                                scalar1=0.0, scalar2=None, op0=ALU.max)
                            v = tmp.tile([128, WINDOW], f16, tag="v")
                            nc.scalar.activation(v[:], d[:], AF.Relu,
                                                 scale=-1.0)
                            fL = fpool.tile([128, WINDOW], f16,
                                            tag=f"f_{2 * k - 2}_{ic}")
                            nc.vector.tensor_tensor(
                                out=fL[:], in0=u[:], in1=q[:], op=ALU.mult)
                            fR = fpool.tile([128, WINDOW], f16,
                                            tag=f"f_{2 * k - 1}_{ic}")
                            nc.vector.tensor_tensor(
                                out=fR[:], in0=v[:], in1=q[:], op=ALU.mult)
                            feat[(2 * k - 2, ic)] = fL
                            feat[(2 * k - 1, ic)] = fR

                    accbs = []
                    for bt in range(N_BT):
                        gbs = slice(w * WINDOW + bt * 128,
                                    w * WINDOW + (bt + 1) * 128)
                        acc_b = psum_b.tile([128, N_OUT], f32, tag="accb")
                        for ic in range(N_IC):
                            nc.tensor.matmul(
                                acc_b[:], sT_tiles[ic][:, gbs], wb_tiles[ic][:],
                                start=(ic == 0), stop=(ic == N_IC - 1))
                        accbs.append(acc_b)
                    for bt in range(N_BT):
                        gbs = slice(w * WINDOW + bt * 128,
                                    w * WINDOW + (bt + 1) * 128)
                        fs = slice(bt * 128, (bt + 1) * 128)
                        acc_s = psum_s.tile([128, N_OUT], f32, tag="accs")
                        ci = 0
                        for fi in range(N_FEAT):
                            for ic in range(N_IC):
                                nc.tensor.matmul(
                                    acc_s[:], feat[(fi, ic)][:, fs],
                                    w2_tiles[fi * N_IC + ic][:],
                                    start=(ci == 0),
                                    stop=(ci == N_FEAT * N_IC - 1))
                                ci += 1
                        silu_t = opool.tile([128, N_OUT], f32, tag="silu")
                        nc.scalar.activation(silu_t[:], accbs[bt][:], AF.Silu)
                        out_t = opool.tile([128, N_OUT], f32, tag="out")
                        nc.vector.tensor_tensor(
                            out=out_t[:], in0=acc_s[:], in1=silu_t[:],
                            op=ALU.add)
                        nc.sync.dma_start(out=out_ext[gbs, :], in_=out_t[:])
    nc.compile()
    return nc


_NC_CACHE = {}


def _get_nc(s_bias, s_hi, loop_n=None):
    key = (s_bias, s_hi, loop_n)
    if key not in _NC_CACHE:
        _NC_CACHE[key] = _build_nc(s_bias, s_hi, loop_n=loop_n)
    return _NC_CACHE[key]


def kernel(x, coeff, base_weight, knots):
    x = np.asarray(x, np.float32)
    assert x.shape == (BATCH, N_IN), x.shape
    w2, wb, inv_h, s_bias, s_hi = _prepare_weights(
        coeff, base_weight, knots)
    nc = _get_nc(s_bias, s_hi)
    sT_list = _prepare_x(x, inv_h)

    in_maps = []
    for c in range(N_CORES):
        in_maps.append({
            "sT": sT_list[c],
            "w2": w2,
            "wb": wb,
        })
    last_err = None
    for attempt in range(3):
        try:
            results = run_bass_kernel_spmd(
                nc, in_maps, list(range(N_CORES))).results
            break
        except Exception as e:  # transient device wedge: retry
            last_err = e
            if attempt == 2:
                raise
            import time
            time.sleep(2.0)
    out = np.concatenate([results[c]["out"] for c in range(N_CORES)], axis=0)
    return out.astype(np.float32)


# revision 5
# speedup vs baseline: 16.7333x; 16.7333x over previous
"""BSplineKAN layer forward on 8 Trainium2 NeuronCores (Bass/Tile).

out = silu(x @ base_weight) + einsum('bir,ior->bo', bspline_basis(x), coeff)

Math: with uniform knots t_j = t3 + (j-3)*h (t3 = left clamp bound) and
s = clamp(x*inv_h + s_bias, 0, G), the cubic B-spline basis functions are
exact linear combinations of 8 one-sided cubic features of s:
  L_k = max(s-k,0)*(s-k)^2 , R_k = min(s-k,0)*(s-k)^2   (k=1..4).
The (features -> basis) map M is solved on the host in float64 and folded
into the coeff tensor:  W2[(f,i), o] = sum_r M[f,r] * coeff[i,o,r].

Host also precomputes sT = transpose(x*inv_h + s_bias) in f16, so the
device needs no transposes or casts: the resident sT tiles serve directly
as (a) the base-matmul stationary operand (base weights are pre-scaled by
h, with the -h*s_bias*sum_i wb[i,o] constant added via a contraction-1
ones-row matmul so it lands inside the silu), and (b) the feature source
after one clamp op.

Per 512-column window the features are built with cheap f16 ops spread
over the three elementwise engines (DVE ts/tt, ACT Square, Pool ts), and
the TensorE does 37 f16 matmuls per 128-row batch tile (32 spline chunks,
4 base chunks, 1 const row). TensorE is the bottleneck at ~31us/window.

Sharding: data-parallel over batch; each of 8 cores handles 2048 rows with
replicated weights. No collectives needed.
"""

import numpy as np

import concourse.bass as bass
import concourse.mybir as mybir
import concourse.tile as tile
from concourse import bacc
from concourse.bass_utils import run_bass_kernel_spmd

N_CORES = 8
BATCH, N_IN, N_OUT = 16384, 512, 512
SPLINE_ORDER, N_GRID = 3, 5
N_BASIS = N_GRID + SPLINE_ORDER  # 8
B_CORE = BATCH // N_CORES        # 2048
N_IC = N_IN // 128               # 4 contraction chunks per feature
N_FEAT = 8
WINDOW = 512
N_W = B_CORE // WINDOW           # 4
N_BT = WINDOW // 128             # 4

f32 = mybir.dt.float32
f16 = mybir.dt.float16
AF = mybir.ActivationFunctionType
ALU = mybir.AluOpType


# ----------------------------------------------------------------------------
# Host-side math
# ----------------------------------------------------------------------------

def _bspline_basis_f64(x, knots):
    """Cox-de Boor recursion (float64), matching the reference semantics."""
    t = np.asarray(knots, np.float64)
    xc = np.clip(np.asarray(x, np.float64),
                 t[SPLINE_ORDER], t[-SPLINE_ORDER - 1])[..., None]
    n_int = len(t) - 1
    B = ((xc >= t[:-1]) & (xc < t[1:])).astype(np.float64)
    for j in range(1, SPLINE_ORDER + 1):
        nv = n_int - j
        ti = t[:nv]
        ti_j = t[j:nv + j]
        ti1 = t[1:nv + 1]
        ti_j1 = t[j + 1:nv + j + 1]
        a1 = (xc - ti) / np.maximum(ti_j - ti, 1e-8)
        a2 = (ti_j1 - xc) / np.maximum(ti_j1 - ti1, 1e-8)
        B = a1 * B[..., :nv] + a2 * B[..., 1:nv + 1]
    return B  # (..., N_BASIS)


def _features_f64(s):
    """One-sided cubes of s (float64). Returns (..., 8)."""
    F = []
    for k in range(1, 5):
        d = s - k
        q = d * d
        F.append(np.maximum(d, 0.0) * q)   # L_k
        F.append(np.minimum(d, 0.0) * q)   # R_k
    return np.stack(F, axis=-1)


def _solve_basis_map(knots):
    """M (8 x 8) with basis = features @ M, solved in f64."""
    t3 = float(knots[SPLINE_ORDER])
    h = float(knots[SPLINE_ORDER + 1] - knots[SPLINE_ORDER])
    g = np.linspace(t3 - 0.5, t3 + N_GRID * h + 0.5, 4001)
    g = np.concatenate([g, np.asarray(knots, np.float64),
                        [t3, t3 + N_GRID * h]])
    sg = (np.clip(g, t3, t3 + N_GRID * h) - t3) / h
    F = _features_f64(sg)
    Bref = _bspline_basis_f64(g, knots)
    M, _, _, _ = np.linalg.lstsq(F, Bref, rcond=None)
    err = np.abs(F @ M - Bref).max()
    # knots come in as float32 and are not exactly uniform, so the closed-form
    # uniform features reproduce the reference basis only to ~1e-7.
    if err > 1e-5:
        raise ValueError(f"basis map residual too large: {err}")
    return M, t3, h


def _prepare_weights(coeff, base_weight, knots):
    """Returns (w2[32,128,512]f16, wb[4,128,512]f16, crow[1,512]f16,
    inv_h, s_bias, s_hi)."""
    M, t3, h = _solve_basis_map(np.asarray(knots, np.float64))
    M = M.copy()
    M[1::2] = -M[1::2]   # device computes fR' = relu(-d)*d^2 = -R_k
    c64 = np.asarray(coeff, np.float64)                      # (i, o, r)
    w2 = np.einsum("fr,ior->fio", M, c64)                    # (8, 512, 512)
    w2_16 = w2.astype(np.float32).astype(np.float16)
    # chunk order [fi*N_IC + ic] -> [32, 128, 512]
    w2_dev = np.ascontiguousarray(
        w2_16.reshape(N_FEAT, N_IC, 128, N_OUT).reshape(-1, 128, N_OUT))
    bw64 = np.asarray(base_weight, np.float64)
    wb_dev = np.ascontiguousarray(
        (h * bw64).astype(np.float32).astype(np.float16)
        .reshape(N_IC, 128, N_OUT))
    inv_h = 1.0 / h
    s_bias = -t3 / h
    s_hi = float((float(knots[-SPLINE_ORDER - 1]) - t3) / h)
    return w2_dev, wb_dev, float(inv_h), float(s_bias), s_hi


def _prepare_x(x, inv_h):
    """Full x (16384, 512) f32 -> per-core s'T arrays [4, 128, 2048] f16,
    with s' = x/h (the -t3/h shift is folded into the device constants so
    the base matmul needs no constant row)."""
    s = (np.asarray(x, np.float32) * np.float32(inv_h)).astype(np.float16)
    # flush f16 subnormals to zero: subnormal stationary operands take a
    # slow path in the PE; |x| < 2.5e-5 contributes nothing to the output
    s = np.where(np.abs(s) < np.float16(6.104e-05), np.float16(0.0), s)
    # (B, I) -> cores (c, b, i) -> [c][ic, p, b]
    s = s.reshape(N_CORES, B_CORE, N_IC, 128)
    sT = np.ascontiguousarray(s.transpose(0, 2, 3, 1))  # (8, 4, 128, 2048)
    return [sT[c] for c in range(N_CORES)]


# ----------------------------------------------------------------------------
# Device kernel (one SPMD program, run on all 8 cores)
# ----------------------------------------------------------------------------

def _build_nc(s_bias, s_hi, loop_n=None):
    """Device tensors hold s' = s - s_bias; clamp bounds and the d_k
    offsets absorb the shift. loop_n wraps the body in a hardware For_i
    loop (idempotent) for delta-timing."""
    nc = bacc.Bacc()
    sT_ext = nc.declare_dram_parameter("sT", [N_IC, 128, B_CORE], f16,
                                       isOutput=False)
    w2_ext = nc.declare_dram_parameter("w2", [N_FEAT * N_IC, 128, N_OUT], f16,
                                       isOutput=False)
    wb_ext = nc.declare_dram_parameter("wb", [N_IC, 128, N_OUT], f16,
                                       isOutput=False)
    out_ext = nc.declare_dram_parameter("out", [B_CORE, N_OUT], f32,
                                        isOutput=True)

    with tile.TileContext(nc) as tc:
        with tc.tile_pool(name="wpool", bufs=1) as wpool, \
             tc.tile_pool(name="spool", bufs=2) as spool, \
             tc.tile_pool(name="xcpool", bufs=2) as xcpool, \
             tc.tile_pool(name="fpool", bufs=2) as fpool, \
             tc.tile_pool(name="tmp", bufs=3) as tmp, \
             tc.tile_pool(name="opool", bufs=3) as opool, \
             tc.tile_pool(name="psum_b", bufs=4, space="PSUM") as psum_b, \
             tc.tile_pool(name="psum_s", bufs=4, space="PSUM") as psum_s:

            # resident weights (outside the timing loop, loaded once)
            w2_tiles = []
            for ci in range(N_FEAT * N_IC):
                t = wpool.tile([128, N_OUT], f16, tag=f"w2_{ci}")
                nc.sync.dma_start(out=t[:], in_=w2_ext[ci])
                w2_tiles.append(t)
            wb_tiles = []
            for ic in range(N_IC):
                t = wpool.tile([128, N_OUT], f16, tag=f"wb_{ic}")
                nc.sync.dma_start(out=t[:], in_=wb_ext[ic])
                wb_tiles.append(t)
            import contextlib
            loop_cm = (tc.For_i(0, loop_n, 1) if loop_n
                       else contextlib.nullcontext())
            with loop_cm:
                sT_tiles = []
                for ic in range(N_IC):
                    t = spool.tile([128, B_CORE], f16, tag=f"sT_{ic}")
                    nc.sync.dma_start(out=t[:], in_=sT_ext[ic])
                    sT_tiles.append(t)

                for w in range(N_W):
                    cs = slice(w * WINDOW, (w + 1) * WINDOW)
                    feat = {}
                    xcs = []
                    for ic in range(N_IC):
                        xc = xcpool.tile([128, WINDOW], f16, tag=f"xc_{ic}")
                        nc.vector.tensor_scalar(
                            out=xc[:], in0=sT_tiles[ic][:, cs],
                            scalar1=-s_bias, scalar2=s_hi - s_bias,
                            op0=ALU.max, op1=ALU.min)
                        xcs.append(xc)
                    for k in range(1, 5):
                        for ic in range(N_IC):
                            d = tmp.tile([128, WINDOW], f16, tag="d")
                            nc.vector.tensor_scalar(
                                out=d[:], in0=xcs[ic][:],
                                scalar1=float(k) - s_bias, scalar2=None,
                                op0=ALU.subtract)
                            q = tmp.tile([128, WINDOW], f16, tag="q")
                            nc.scalar.activation(q[:], d[:], AF.Square)
                            u = tmp.tile([128, WINDOW], f16, tag="u")
                            nc.vector.tensor_scalar(
                                out=u[:], in0=d[:],
                                scalar1=0.0, scalar2=None, op0=ALU.max)
                            v = tmp.tile([128, WINDOW], f16, tag="v")
                            nc.scalar.activation(v[:], d[:], AF.Relu,
                                                 scale=-1.0)
                            fL = fpool.tile([128, WINDOW], f16,
                                            tag=f"f_{2 * k - 2}_{ic}")
                            nc.vector.tensor_tensor(
                                out=fL[:], in0=u[:], in1=q[:], op=ALU.mult)
                            fR = fpool.tile([128, WINDOW], f16,
                                            tag=f"f_{2 * k - 1}_{ic}")
                            nc.vector.tensor_tensor(
                                out=fR[:], in0=v[:], in1=q[:], op=ALU.mult)
                            feat[(2 * k - 2, ic)] = fL
                            feat[(2 * k - 1, ic)] = fR

                    accbs = []
                    for bt in range(N_BT):
                        gbs = slice(w * WINDOW + bt * 128,
                                    w * WINDOW + (bt + 1) * 128)
                        acc_b = psum_b.tile([128, N_OUT], f32, tag="accb")
                        for ic in range(N_IC):
                            nc.tensor.matmul(
                                acc_b[:], sT_tiles[ic][:, gbs], wb_tiles[ic][:],
                                start=(ic == 0), stop=(ic == N_IC - 1))
                        accbs.append(acc_b)
                    for bt in range(N_BT):
                        gbs = slice(w * WINDOW + bt * 128,
                                    w * WINDOW + (bt + 1) * 128)
                        fs = slice(bt * 128, (bt + 1) * 128)
                        acc_s = psum_s.tile([128, N_OUT], f32, tag="accs")
                        ci = 0
                        for fi in range(N_FEAT):
                            for ic in range(N_IC):
                                nc.tensor.matmul(
                                    acc_s[:], feat[(fi, ic)][:, fs],
                                    w2_tiles[fi * N_IC + ic][:],
                                    start=(ci == 0),
                                    stop=(ci == N_FEAT * N_IC - 1))
                                ci += 1
                        silu_t = opool.tile([128, N_OUT], f32, tag="silu")
                        nc.scalar.activation(silu_t[:], accbs[bt][:], AF.Silu)
                        out_t = opool.tile([128, N_OUT], f32, tag="out")
                        nc.vector.tensor_tensor(
                            out=out_t[:], in0=acc_s[:], in1=silu_t[:],
                            op=ALU.add)
                        nc.sync.dma_start(out=out_ext[gbs, :], in_=out_t[:])
    nc.compile()
    return nc


_NC_CACHE = {}


def _get_nc(s_bias, s_hi, loop_n=None):
    key = (s_bias, s_hi, loop_n)
    if key not in _NC_CACHE:
        _NC_CACHE[key] = _build_nc(s_bias, s_hi, loop_n=loop_n)
    return _NC_CACHE[key]


def kernel(x, coeff, base_weight, knots):
    x = np.asarray(x, np.float32)
    assert x.shape == (BATCH, N_IN), x.shape
    w2, wb, inv_h, s_bias, s_hi = _prepare_weights(
        coeff, base_weight, knots)
    nc = _get_nc(s_bias, s_hi)
    sT_list = _prepare_x(x, inv_h)

    in_maps = []
    for c in range(N_CORES):
        in_maps.append({
            "sT": sT_list[c],
            "w2": w2,
            "wb": wb,
        })
    last_err = None
    for attempt in range(3):
        try:
            results = run_bass_kernel_spmd(
                nc, in_maps, list(range(N_CORES))).results
            break
        except Exception as e:  # transient device wedge: retry
            last_err = e
            if attempt == 2:
                raise
            import time
            time.sleep(2.0)
    out = np.concatenate([results[c]["out"] for c in range(N_CORES)], axis=0)
    return out.astype(np.float32)
